# revision 50
# baseline (speedup 1.0000x reference)
"""Distributed Trainium2 kernel for nn_Attention (RMSNorm + QKV + RoPE +
causal SDPA + out-proj) over 8 NeuronCores.

v6.1 strategy (batch x head-group sharding, collective-free start):
  Each core owns (batch b = c//4, heads 4g..4g+3 where g = c%4) and gets
  its batch's full activations xT as input -- there is NO AllGather and
  no startup barrier on the critical path (a tiny dummy AllGather issued
  at t~0 absorbs the one-time global rendezvous in the background).
  RMSNorm is applied lazily: raw projections q = Wq^T x are scaled by
  1/rms(token) post-matmul (scale commutes with the projection; norm_w
  is folded into the staged weights).  Per 512-token chunk: x tiles are
  DMA'd fp32, cast to bf16 (DVE), squared (DVE, bf16), ones-reduced on
  PE into ssq, Sqrt+recip -> inv; inv is broadcast to a [128,1024]
  pair-duplicated tile via outer-product matmuls for q/k scaling and
  transposed to a per-token column via tiny matmuls for V scaling.
  q/k use a pair-interleaved layout (chunk-major, pair within chunk) so
  every RoPE elementwise op is 1024 wide.  SDPA steps (pair, q-tile)
  interleave with the projections (row-packed K=64 score matmuls on the
  two array halves, one exp per block covering both heads, ones-column
  in V for the softmax denominator, deferred divide; diag masking on
  GpSimd).  Tail: one 8-rank AllToAll where slot j carries my 256 ctx
  dims x rank j's 256-token sub-chunk of my batch ([1024,512] layout,
  zero waste), PE warm-keeper matmuls across the A2A window, then the
  out-projection for the core's 2x256-token output chunk.
Host does layout-only prep (transpose, per-core head-column slices,
constant RoPE/mask tables) and the final scatter.
"""
import sys

sys.path.insert(0, "/opt/trn_rl_repo")

import numpy as np
import ml_dtypes
from contextlib import ExitStack

import concourse.bass as bass
import concourse.mybir as mybir
import concourse.tile as tile
from concourse import bacc
from concourse.bass_utils import run_bass_kernel_spmd

F32 = mybir.dt.float32
BF16 = mybir.dt.bfloat16

B, S, D, H, DH = 2, 2048, 1024, 16, 64
NC = 8
CHUNK = 512
NCH = S // CHUNK       # chunks per batch: 4
NKB = S // 128         # key blocks per batch: 16
QT = S // CHUNK        # q tiles per batch: 4
EPS = 1.1920929e-07
THETA = 10000.0
NKEEP = 12             # PE warm-keeper matmuls bridging the pair-1 A2A

_CACHE = {}
DEBUG_DUMP = False


def _build():
    nc = bacc.Bacc("TRN2", target_bir_lowering=False, debug=False, num_devices=NC)

    xt_d = nc.dram_tensor("xt", [D, S], F32, kind="ExternalInput")
    nw_d = nc.dram_tensor("nw", [D, 1], F32, kind="ExternalInput")
    wq_d = nc.dram_tensor("wqc", [D, 256], F32, kind="ExternalInput")
    wk_d = nc.dram_tensor("wkc", [D, 256], F32, kind="ExternalInput")
    wv_d = nc.dram_tensor("wvc", [D, 256], F32, kind="ExternalInput")
    wo_d = nc.dram_tensor("wo", [D, D], F32, kind="ExternalInput")
    cos_d = nc.dram_tensor("cosb", [128, S], BF16, kind="ExternalInput")
    sin_d = nc.dram_tensor("sinb", [128, S], BF16, kind="ExternalInput")
    msk_d = nc.dram_tensor("dmask", [128, 128], BF16, kind="ExternalInput")
    pm_d = nc.dram_tensor("permm", [128, 128], BF16, kind="ExternalInput")
    id_d = nc.dram_tensor("identm", [128, 128], BF16, kind="ExternalInput")
    out_d = nc.dram_tensor("out", [CHUNK, D], F32, kind="ExternalOutput")

    dmy_in = nc.dram_tensor("dmy_in", [1, 64], BF16)
    dmy_out = nc.dram_tensor("dmy_out", [NC, 64], BF16, addr_space="Shared")
    # per-pair AllToAll: slot j (rows [j*128,(j+1)*128)) = my 128 ctx dims
    # of pair p x rank j's 256-token sub-chunk of my batch; after the
    # exchange, (row i*128+r, col t) holds ctx dim r of pair p of rank i
    # for my token t -- uniform on all cores.  Pair 0's exchange triggers
    # before the final (pair 1) SDPA step and overlaps it.
    a2aA_in = nc.dram_tensor("a2aA_in", [NC * 128, 256], BF16)
    a2aA_out = nc.dram_tensor("a2aA_out", [NC * 128, 256], BF16)
    a2aB_in = nc.dram_tensor("a2aB_in", [NC * 128, 256], BF16)
    a2aB_out = nc.dram_tensor("a2aB_out", [NC * 128, 256], BF16)

    if DEBUG_DUMP:
        dbg_q = nc.dram_tensor("dbg_q", [128, 2 * S], BF16, kind="ExternalOutput")
        dbg_k = nc.dram_tensor("dbg_k", [128, 2 * S], BF16, kind="ExternalOutput")
        dbg_v = nc.dram_tensor("dbg_v", [128, 2 * NKB * 130], BF16,
                               kind="ExternalOutput")
        dbg_c = nc.dram_tensor("dbg_c", [128, 2 * S], BF16, kind="ExternalOutput")
        dbg_i = nc.dram_tensor("dbg_i", [128, 2 * S], BF16, kind="ExternalOutput")
        dbg_ao = nc.dram_tensor("dbg_ao", [NC * 128, 256], BF16,
                                kind="ExternalOutput")

    with tile.TileContext(nc) as tc, ExitStack() as ctx:
        pp = ctx.enter_context(tc.tile_pool(name="persist", bufs=1))

        # ---- persistent tiles ----
        # q/k layout: chunk-major, pair within chunk: col = i*1024 + p*512 + t
        qT = pp.tile([128, 2 * S], BF16, tag="qT")
        kT = pp.tile([128, 2 * S], BF16, tag="kT")
        v_all = pp.tile([128, 2 * NKB * 130], BF16, tag="v_all")
        dmsk = pp.tile([128, 128], BF16, tag="dmsk")   # additive bias: 0 / -240
        identm = pp.tile([128, 128], BF16, tag="identm")
        ones1 = pp.tile([1, 128], BF16, tag="ones1")
        ones128 = pp.tile([128, 1], BF16, tag="ones128")
        nw_sb = pp.tile([128, 8], F32, tag="nw_sb")
        permT = pp.tile([128, 128], BF16, tag="permT")
        wq_sb = pp.tile([128, 8, 256], BF16, tag="wq_sb")
        wk_sb = pp.tile([128, 8, 256], BF16, tag="wk_sb")
        wv_sb = pp.tile([128, 8, 256], BF16, tag="wv_sb")
        wo_sb = pp.tile([128, 8, 1024], BF16, tag="wo_sb")
        ctx_sb = pp.tile([128, 2 * S], BF16, tag="ctx_sb")   # pair p at p*S
        invB_sb = pp.tile([128, 2 * S], BF16, tag="invB_sb")  # chunk i at i*1024
        invT_sb = pp.tile([128, 16], F32, tag="invT_sb")   # chunk i at 4i..4i+4
        eps_t = pp.tile([1, 1], F32, tag="eps_t")
        dmy_sb = pp.tile([1, 64], BF16, tag="dmy_sb")
        vv4 = v_all.rearrange("p (pr blk h c) -> p pr blk h c", pr=2, blk=NKB, c=65)

        nc.vector.memset(ones1, 1.0)
        nc.vector.memset(ones128, 1.0)
        nc.vector.memset(eps_t, float(EPS))
        nc.vector.memset(dmy_sb, 0.0)
        # dummy collective: absorbs the one-time global barrier in the
        # background while phase B computes
        nc.sync.dma_start(out=dmy_in[:, :], in_=dmy_sb)
        nc.gpsimd.collective_compute(
            "AllGather", mybir.AluOpType.bypass,
            replica_groups=[list(range(NC))],
            ins=[dmy_in.ap().opt()], outs=[dmy_out.ap().opt()])
        # ones columns of the extended V blocks (softmax denominator)
        nc.gpsimd.memset(vv4[:, :, :, :, 64:65], 1.0)
        nc.sync.dma_start(out=nw_sb.rearrange("p (kt o) -> p kt o", o=1),
                          in_=nw_d.rearrange("(kt p) o -> p kt o", p=128))

        # ---- pools ----
        sc3 = nc.named_scope("p3_projsdpa"); sc3.__enter__()
        pb_cm = tc.tile_pool(name="pb", bufs=2)
        pb_pool = pb_cm.__enter__()
        pexp_cm = tc.tile_pool(name="pexp", bufs=4)
        pexp = pexp_cm.__enter__()
        cn_cm = tc.tile_pool(name="cn", bufs=2)
        cn = cn_cm.__enter__()
        cs_cm = tc.tile_pool(name="cspool", bufs=2)
        cs_pool = cs_cm.__enter__()
        big_cm = tc.tile_pool(name="bigp", bufs=2, space="PSUM")
        big_pool = big_cm.__enter__()
        ctx_cm = tc.tile_pool(name="ctxp", bufs=1, space="PSUM")
        ctx_pool = ctx_cm.__enter__()
        wo_cm = tc.tile_pool(name="wostage", bufs=2)
        wostage = wo_cm.__enter__()
        xf_cm = tc.tile_pool(name="xf", bufs=8)
        xf_pool = xf_cm.__enter__()
        xc_cm = tc.tile_pool(name="xc", bufs=2)
        xc_pool = xc_cm.__enter__()
        xq_cm = tc.tile_pool(name="xq", bufs=2)
        xq_pool = xq_cm.__enter__()
        ssq_cm = tc.tile_pool(name="ssqp", bufs=1, space="PSUM")
        ssq_pool = ssq_cm.__enter__()

        xcs = {}

        def emit_load(i):
            """DMA + cast + square for chunk i (no PE work)."""
            xc = xc_pool.tile([128, 8, CHUNK], BF16, name="xc", tag="xc")
            xq = xq_pool.tile([128, 8, CHUNK], BF16, name="xq", tag="xq")
            cst = cs_pool.tile([128, 1024], BF16, name="cst", tag="cst")
            snt = cs_pool.tile([128, 1024], BF16, name="snt", tag="snt")
            xcs[i] = (xc, xq, cst, snt)
            for p in (0, 1):
                nc.sync.dma_start(out=cst[:, p * 512 : (p + 1) * 512],
                                  in_=cos_d[:, i * CHUNK : (i + 1) * CHUNK])
                nc.sync.dma_start(out=snt[:, p * 512 : (p + 1) * 512],
                                  in_=sin_d[:, i * CHUNK : (i + 1) * CHUNK])
            for kt in range(8):
                xf = xf_pool.tile([128, CHUNK], F32, name="xf", tag="xf")
                nc.sync.dma_start(
                    out=xf, in_=xt_d[kt * 128 : (kt + 1) * 128,
                                     i * CHUNK : (i + 1) * CHUNK])
                nc.vector.tensor_copy(xc[:, kt, :], xf)
                nc.gpsimd.tensor_mul(xq[:, kt, :], xc[:, kt, :], xc[:, kt, :])

        def stage_w(w_sb, d):
            st = wstage.tile([128, 8 * 256], F32, tag="wst")
            nc.sync.dma_start(out=st.rearrange("p (kt c) -> p kt c", c=256),
                              in_=d.rearrange("(kt p) c -> p kt c", p=128))
            for kt in range(8):
                nc.vector.tensor_scalar(
                    out=w_sb[:, kt, :], in0=st[:, kt * 256 : (kt + 1) * 256],
                    scalar1=nw_sb[:, kt : kt + 1], scalar2=None,
                    op0=mybir.AluOpType.mult)

        def emit_inv(i):
            """ssq-reduce -> inv row (pair-duplicated bcast) + inv column."""
            xq = xcs[i][1]
            ssq = ssq_pool.tile([1, CHUNK], F32, name="ssq", tag="ssq")
            for kt in range(8):
                nc.tensor.matmul(ssq, ones128, xq[:, kt, :],
                                 start=(kt == 0), stop=(kt == 7))
            rstd = cn.tile([1, CHUNK], F32, name="rstd", tag="rstd")
            nc.scalar.activation(rstd, ssq, mybir.ActivationFunctionType.Sqrt,
                                 bias=eps_t[0:1, 0:1], scale=1.0 / D)
            inv = cn.tile([1, CHUNK], F32, name="inv", tag="inv")
            nc.vector.reciprocal_approx_fast(out=inv, in_=rstd)
            invb = cn.tile([1, CHUNK], BF16, name="invb", tag="invb")
            nc.vector.tensor_copy(invb, inv)
            bigt = big_pool.tile([128, 1024], F32, name="ibp", tag="big")
            nc.tensor.matmul(bigt[:, 0:512], ones1, invb, start=True, stop=True)
            nc.tensor.matmul(bigt[:, 512:1024], ones1, invb, start=True, stop=True)
            nc.vector.tensor_copy(invB_sb[:, i * 1024 : (i + 1) * 1024], bigt)
            bigt2 = big_pool.tile([128, 1024], F32, name="itp", tag="big")
            for tb in range(4):
                nc.tensor.matmul(bigt2[:, tb : tb + 1],
                                 invb[0:1, tb * 128 : (tb + 1) * 128],
                                 ones1[0:1, 0:1], start=True, stop=True)
            nc.vector.tensor_copy(invT_sb[:, i * 4 : (i + 1) * 4], bigt2[:, 0:4])

        def proj_chunk(i):
            """all projection matmuls first (PE never waits on DVE), then
            the rope finishes and V scales."""
            xc, _, cst, snt = xcs[i]
            c0 = i * 1024
            accs, ts, swps = {}, {}, {}
            for name, w_sb in (("q", wq_sb), ("k", wk_sb)):
                bigt = big_pool.tile([128, 1024], F32, name="a" + name, tag="big")
                accs[name] = bigt
                for p in (0, 1):
                    acc = bigt[:, p * 512 : (p + 1) * 512]
                    for kt in range(8):
                        nc.tensor.matmul(acc, w_sb[:, kt, p * 128 : (p + 1) * 128],
                                         xc[:, kt, :], start=(kt == 0),
                                         stop=(kt == 7))
                t = pb_pool.tile([128, 1024], BF16, tag="rt")
                nc.vector.tensor_mul(t, bigt,
                                     invB_sb[:, i * 1024 : (i + 1) * 1024])
                ts[name] = t
            for name in ("q", "k"):
                swpt = big_pool.tile([128, 1024], F32, name="s" + name, tag="big")
                swps[name] = swpt
                t = ts[name]
                nc.tensor.matmul(swpt[:, 0:512], permT, t[:, 0:512],
                                 start=True, stop=True)
                nc.tensor.matmul(swpt[:, 512:1024], permT, t[:, 512:1024],
                                 start=True, stop=True)
            # V token-major: [128 tokens, 256 dims] per 128-token block, all
            # four blocks in one PSUM tile
            vtile = big_pool.tile([128, 1024], F32, name="vb", tag="big")
            for tb in range(4):
                vp = vtile[:, tb * 256 : (tb + 1) * 256]
                for kt in range(8):
                    nc.tensor.matmul(
                        vp, xc[:, kt, tb * 128 : (tb + 1) * 128],
                        wv_sb[:, kt, :], start=(kt == 0), stop=(kt == 7))
            # rope finishes (DVE) -- overlap the V matmuls
            for name, dst in (("q", qT), ("k", kT)):
                t, swpt = ts[name], swps[name]
                sw = pb_pool.tile([128, 1024], BF16, tag="rsw")
                nc.vector.tensor_mul(sw, swpt, snt)
                nc.vector.tensor_mul(t, t, cst)
                nc.vector.tensor_add(dst[:, c0 : c0 + 1024], t, sw)
            # V scale per token block (per-partition scalar)
            for tb in range(4):
                blk = i * 4 + tb
                vp = vtile[:, tb * 256 : (tb + 1) * 256]
                src = vp.rearrange("p (pr h c) -> p pr h c", pr=2, c=64)
                nc.vector.tensor_scalar(
                    out=vv4[:, :, blk, :, 0:64], in0=src,
                    scalar1=invT_sb[:, blk : blk + 1], scalar2=None,
                    op0=mybir.AluOpType.mult)

        def emit_norm(p, j, ctxp):
            base = p * S
            for hi, r0 in ((0, 0), (1, 64)):
                den_s = cn.tile([1, CHUNK], F32, tag="den_s")
                nc.vector.tensor_copy(den_s, ctxp[hi][64:65, :])
                rec = cn.tile([1, CHUNK], F32, tag="rec")
                nc.vector.reciprocal_approx_fast(out=rec, in_=den_s)
                recb = cn.tile([1, CHUNK], BF16, tag="recb")
                nc.vector.tensor_copy(recb, rec)
                bigt = big_pool.tile([128, 1024], F32, name="bc", tag="big")
                nc.tensor.matmul(bigt[0:64, 0:CHUNK], ones1[0:1, 0:64], recb,
                                 start=True, stop=True)
                bcs = cn.tile([64, CHUNK], BF16, tag="bcs")
                nc.vector.tensor_copy(bcs, bigt[0:64, 0:CHUNK])
                nc.vector.tensor_mul(
                    ctx_sb[r0 : r0 + 64, base + CHUNK * j : base + CHUNK * (j + 1)],
                    ctxp[hi][0:64, :], bcs)
            a2a_d = a2aA_in if p == 0 else a2aB_in
            for half in (0, 1):
                slot = (j * 2 + half) * 128
                col0 = base + j * CHUNK + half * 256
                nc.sync.dma_start(
                    out=a2a_d[slot : slot + 128, :],
                    in_=ctx_sb[:, col0 : col0 + 256])

        def emit_block(p, j, kb, ctxp, sc_pools=None):
            nkb = 4 * (j + 1)
            m = kb - 4 * j
            c0 = 128 * m if m >= 0 else 0
            w = CHUNK - c0
            qcol0 = j * 1024 + p * 512 + c0
            koff = (kb // 4) * 1024 + p * 512 + (kb % 4) * 128
            pool = big_pool if sc_pools is None else sc_pools[kb % len(sc_pools)]
            sc = pool.tile([128, 1024], F32, name="sc", tag="big")
            for hi, r0 in ((0, 0), (1, 64)):
                nc.tensor.matmul(
                    sc[:, hi * 512 + c0 : hi * 512 + 512],
                    kT[r0 : r0 + 64, koff : koff + 128],
                    qT[r0 : r0 + 64, qcol0 : qcol0 + w],
                    start=True, stop=(m < 0), skip_group_check=True)
                if m >= 0:
                    # diag block: accumulate the causal bias (-240 above the
                    # diagonal) into the 128-col window via identity matmul
                    nc.tensor.matmul(
                        sc[:, hi * 512 + c0 : hi * 512 + c0 + 128],
                        identm, dmsk, start=False, stop=True,
                        skip_group_check=True)
            pt = pexp.tile([128, 1024], BF16, name="pt", tag="pt")
            nc.scalar.activation(pt[:, c0:1024], sc[:, c0:1024],
                                 mybir.ActivationFunctionType.Exp, scale=0.125)
            vcol = (p * NKB + kb) * 130
            for hi in range(2):
                nc.tensor.matmul(
                    ctxp[hi][:, c0:CHUNK],
                    v_all[:, vcol + hi * 65 : vcol + hi * 65 + 65],
                    pt[:, hi * 512 + c0 : hi * 512 + 512],
                    start=(kb == 0), stop=(kb == nkb - 1),
                    skip_group_check=True)

        def emit_step(p, j, sc_pools=None):
            ctxp = {0: ctx_pool.tile([65, CHUNK], F32, name="ctxA", tag="ctxA"),
                    1: ctx_pool.tile([65, CHUNK], F32, name="ctxB", tag="ctxB")}
            for kb in range(4 * (j + 1)):
                emit_block(p, j, kb, ctxp, sc_pools)
            emit_norm(p, j, ctxp)

        def stage_wo(kt):
            st2 = wostage.tile([128, 1024], F32, tag="wost")
            nc.sync.dma_start(out=st2[0:64, :], in_=wo_d[kt * 128 : kt * 128 + 64, :])
            nc.sync.dma_start(out=st2[64:128, :],
                              in_=wo_d[kt * 128 + 64 : (kt + 1) * 128, :])
            nc.vector.tensor_copy(wo_sb[:, kt, :], st2)

        # ---- startup: x chunk 0 first, then weights/tables by need-time ----
        emit_load(0)
        wst_cm = tc.tile_pool(name="wstage", bufs=2)
        wstage = wst_cm.__enter__()
        stage_w(wq_sb, wq_d)
        stage_w(wk_sb, wk_d)
        stage_w(wv_sb, wv_d)
        wst_cm.__exit__(None, None, None)
        nc.sync.dma_start(out=dmsk, in_=msk_d[:, :])
        nc.sync.dma_start(out=permT, in_=pm_d[:, :])
        nc.sync.dma_start(out=identm, in_=id_d[:, :])

        # steps run as their chunk prerequisites complete; chunk i+1's
        # load + inv chain is emitted inside chunk i's step stream so the
        # Sqrt/recip/broadcast latency hides behind the step matmuls
        steps_by_chunk = {0: [(0, 0)], 1: [(1, 0), (0, 1)],
                          2: [(1, 1), (0, 2)], 3: [(1, 2), (0, 3)]}
        emit_inv(0)
        for i in range(NCH):
            proj_chunk(i)
            stage_wo(2 * i)
            stage_wo(2 * i + 1)
            if i + 1 < NCH:
                emit_load(i + 1)   # casts/squares overlap this chunk's steps
            first = True
            for (p, j) in steps_by_chunk[i]:
                emit_step(p, j)
                if first and i + 1 < NCH:
                    emit_inv(i + 1)
                    first = False

        # pair-0 context is complete: launch its AllToAll now so the wire
        # time hides behind the final (pair 1) step
        sc4 = nc.named_scope("p4_a2aA"); sc4.__enter__()
        nc.gpsimd.collective_compute(
            "AllToAll", mybir.AluOpType.bypass,
            replica_groups=[list(range(NC))],
            ins=[a2aA_in.ap().opt()], outs=[a2aA_out.ap().opt()])
        sc4.__exit__(None, None, None)

        # release the x/ssq pools, run the final step with a 3-deep
        # score-buffer rotation (big,big,sc3)
        ssq_cm.__exit__(None, None, None)
        xq_cm.__exit__(None, None, None)
        xc_cm.__exit__(None, None, None)
        xf_cm.__exit__(None, None, None)
        sc3p_cm = tc.tile_pool(name="sc3p", bufs=1, space="PSUM")
        sc3p = sc3p_cm.__enter__()
        emit_step(1, 3, sc_pools=[big_pool, big_pool, sc3p])

        sc3p_cm.__exit__(None, None, None)
        wo_cm.__exit__(None, None, None)
        ctx_cm.__exit__(None, None, None)
        big_cm.__exit__(None, None, None)
        cs_cm.__exit__(None, None, None)
        cn_cm.__exit__(None, None, None)
        pexp_cm.__exit__(None, None, None)
        pb_cm.__exit__(None, None, None)
        sc3.__exit__(None, None, None)

        # ---- A2A (pair 1) + out-projection ----
        # ctx dim-tile u (0..7) = pair u%2 of group-rank u//2 -> rows
        # (h*4 + u//2)*128 of a2a{A,B}_out for batch h
        sc6 = nc.named_scope("p6_outproj"); sc6.__enter__()
        with tc.tile_pool(name="ctxgp", bufs=1) as ctxgp, \
             tc.tile_pool(name="outp", bufs=2) as outp, \
             tc.tile_pool(name="ps6", bufs=2, space="PSUM") as ps6, \
             tc.tile_pool(name="keep", bufs=1, space="PSUM") as keepp:
            sc5 = nc.named_scope("p5_a2aB"); sc5.__enter__()
            nc.gpsimd.collective_compute(
                "AllToAll", mybir.AluOpType.bypass,
                replica_groups=[list(range(NC))],
                ins=[a2aB_in.ap().opt()], outs=[a2aB_out.ap().opt()])
            sc5.__exit__(None, None, None)

            ctxg = ctxgp.tile([128, 8, 2, 256], BF16, tag="ctxg")
            for pr, a2a_o in ((0, a2aA_out), (1, a2aB_out)):
                for h in range(2):
                    for gi in range(4):
                        r0 = (h * 4 + gi) * 128
                        nc.sync.dma_start(out=ctxg[:, 2 * gi + pr, h, :],
                                          in_=a2a_o[r0 : r0 + 128, :])

            def op_mms(pso, h, tb2, us, start, stop):
                for n, u in enumerate(us):
                    stat = ctxg[:, u, h, tb2 * 128 : (tb2 + 1) * 128]
                    st = start and n == 0
                    sp = stop and n == len(us) - 1
                    nc.tensor.matmul(pso[:, 0:512], stat, wo_sb[:, u, 0:512],
                                     start=st, stop=sp, skip_group_check=True)
                    nc.tensor.matmul(pso[:, 512:1024], stat,
                                     wo_sb[:, u, 512:1024],
                                     start=st, stop=sp, skip_group_check=True)

            def flush(pso, h, tb2):
                ost = outp.tile([128, 1024], F32, tag="ost")
                nc.scalar.copy(ost[:, 0:512], pso[:, 0:512])
                nc.scalar.copy(ost[:, 512:1024], pso[:, 512:1024])
                r0 = h * 256 + tb2 * 128
                nc.sync.dma_start(out=out_d[r0 : r0 + 64, :], in_=ost[0:64, :])
                nc.sync.dma_start(out=out_d[r0 + 64 : r0 + 128, :],
                                  in_=ost[64:128, :])

            # batch-0 halves: pair-0 contributions run during the pair-1
            # A2A (warm-keeping the PE), then keepers bridge the rest
            pso0 = {tb2: ps6.tile([128, 1024], F32, name=f"ps0{tb2}", tag="pso")
                    for tb2 in range(2)}
            for tb2 in range(2):
                op_mms(pso0[tb2], 0, tb2, [0, 2, 4, 6], True, False)
            keep = keepp.tile([128, 512], F32, tag="keep")
            for _ in range(NKEEP):
                nc.tensor.matmul(keep, wo_sb[:, 0, 0:128], wo_sb[:, 1, 0:512],
                                 start=True, stop=True)
            for tb2 in range(2):
                op_mms(pso0[tb2], 0, tb2, [1, 3, 5, 7], False, True)
                flush(pso0[tb2], 0, tb2)
            for tb2 in range(2):
                pso = ps6.tile([128, 1024], F32, name=f"ps1{tb2}", tag="pso")
                op_mms(pso, 1, tb2, [0, 2, 4, 6, 1, 3, 5, 7], True, True)
                flush(pso, 1, tb2)
        sc6.__exit__(None, None, None)

        if DEBUG_DUMP:
            nc.sync.dma_start(out=dbg_q[:, :], in_=qT)
            nc.sync.dma_start(out=dbg_k[:, :], in_=kT)
            nc.sync.dma_start(out=dbg_v[:, :], in_=v_all)
            nc.sync.dma_start(out=dbg_c[:, :], in_=ctx_sb)
            nc.sync.dma_start(out=dbg_i[:, :], in_=invB_sb)
            nc.sync.dma_start(out=dbg_ao[:, :], in_=a2aA_out[:, :])

    nc.compile()
    return nc


def _head_cols(h, deinterleave):
    base = h * DH
    if deinterleave:
        return np.concatenate([base + np.arange(0, DH, 2), base + np.arange(1, DH, 2)])
    return base + np.arange(DH)


def _make_tables():
    inv_freq = 1.0 / (THETA ** (np.arange(0, DH, 2) / DH))   # [32]
    ang = np.arange(S)[:, None] * inv_freq[None, :]          # [2048, 32]
    ch = np.cos(ang).T.astype(np.float32)                    # [32, 2048]
    sh = np.sin(ang).T.astype(np.float32)
    cosb = np.concatenate([ch, ch, ch, ch], axis=0)          # [128, 2048]
    sinb = np.concatenate([-sh, sh, -sh, sh], axis=0)
    kk, qq = np.meshgrid(np.arange(128), np.arange(128), indexing="ij")
    dmask = np.where(kk <= qq, 0.0, -240.0).astype(np.float32)
    bf = ml_dtypes.bfloat16
    return cosb.astype(bf), sinb.astype(bf), dmask.astype(bf)


def _in_maps(inputs):
    x = np.ascontiguousarray(inputs["x"], dtype=np.float32)
    norm_w = np.asarray(inputs["norm_w"], dtype=np.float32)
    wq = np.asarray(inputs["wq"], dtype=np.float32)
    wk = np.asarray(inputs["wk"], dtype=np.float32)
    wv = np.asarray(inputs["wv"], dtype=np.float32)
    wo = np.ascontiguousarray(inputs["wo"], dtype=np.float32)

    xT = [np.ascontiguousarray(x[b].T) for b in range(B)]    # [1024, 2048] each
    cosb, sinb, dmask = _make_tables()
    nw = np.ascontiguousarray(norm_w.reshape(D, 1))
    # rotate-half partition swap as a (symmetric) permutation matrix
    swap = np.concatenate([np.arange(32, 64), np.arange(0, 32),
                           np.arange(96, 128), np.arange(64, 96)])
    perm = np.zeros((128, 128), np.float32)
    perm[np.arange(128), swap] = 1.0
    perm = perm.astype(ml_dtypes.bfloat16)
    ident = np.eye(128, dtype=np.float32).astype(ml_dtypes.bfloat16)

    maps = []
    for c in range(NC):
        b, g = c // 4, c % 4
        heads = [4 * g + 0, 4 * g + 1, 4 * g + 2, 4 * g + 3]
        qcols = np.concatenate([_head_cols(h, True) for h in heads])
        vcols = np.concatenate([_head_cols(h, False) for h in heads])
        maps.append({
            "xt": xT[b],
            "nw": nw,
            "wqc": np.ascontiguousarray(wq[:, qcols]),
            "wkc": np.ascontiguousarray(wk[:, qcols]),
            "wvc": np.ascontiguousarray(wv[:, vcols]),
            "wo": wo,
            "cosb": cosb,
            "sinb": sinb,
            "dmask": dmask,
            "permm": perm,
            "identm": ident,
        })
    return maps


def _run(inputs, trace=False):
    if "ncs" not in _CACHE:
        _CACHE["ncs"] = _build()
    nc = _CACHE["ncs"]
    res = run_bass_kernel_spmd(nc, _in_maps(inputs), core_ids=list(range(NC)),
                               trace=trace)
    out = np.empty((B, S, D), dtype=np.float32)
    for c in range(NC):
        chunk = res.results[c]["out"]        # [512, D]: rows 0-255 batch 0,
        out[0, c * 256 : (c + 1) * 256] = chunk[0:256]    # 256-511 batch 1
        out[1, c * 256 : (c + 1) * 256] = chunk[256:512]
    return out, res


def kernel(**inputs) -> np.ndarray:
    out, _ = _run(inputs, trace=False)
    return out


# revision 52
# speedup vs baseline: 1.1092x; 1.1092x over previous
"""Distributed Trainium2 kernel for nn_Attention (RMSNorm + QKV + RoPE +
causal SDPA + out-proj) over 8 NeuronCores.

v6.1 strategy (batch x head-group sharding, collective-free start):
  Each core owns (batch b = c//4, heads 4g..4g+3 where g = c%4) and gets
  its batch's full activations xT as input -- there is NO AllGather and
  no startup barrier on the critical path (a tiny dummy AllGather issued
  at t~0 absorbs the one-time global rendezvous in the background).
  RMSNorm is applied lazily: raw projections q = Wq^T x are scaled by
  1/rms(token) post-matmul (scale commutes with the projection; norm_w
  is folded into the staged weights).  Per 512-token chunk: x tiles are
  DMA'd fp32, cast to bf16 (DVE), squared (DVE, bf16), ones-reduced on
  PE into ssq, Sqrt+recip -> inv; inv is broadcast to a [128,1024]
  pair-duplicated tile via outer-product matmuls for q/k scaling and
  transposed to a per-token column via tiny matmuls for V scaling.
  q/k use a pair-interleaved layout (chunk-major, pair within chunk) so
  every RoPE elementwise op is 1024 wide.  SDPA steps (pair, q-tile)
  interleave with the projections (row-packed K=64 score matmuls on the
  two array halves, one exp per block covering both heads, ones-column
  in V for the softmax denominator, deferred divide; diag masking on
  GpSimd).  Tail: one 8-rank AllToAll where slot j carries my 256 ctx
  dims x rank j's 256-token sub-chunk of my batch ([1024,512] layout,
  zero waste), PE warm-keeper matmuls across the A2A window, then the
  out-projection for the core's 2x256-token output chunk.
Host does layout-only prep (transpose, per-core head-column slices,
constant RoPE/mask tables) and the final scatter.
"""
import sys

sys.path.insert(0, "/opt/trn_rl_repo")

import numpy as np
import ml_dtypes
from contextlib import ExitStack

import concourse.bass as bass
import concourse.mybir as mybir
import concourse.tile as tile
from concourse import bacc
from concourse.bass_utils import run_bass_kernel_spmd

F32 = mybir.dt.float32
BF16 = mybir.dt.bfloat16

B, S, D, H, DH = 2, 2048, 1024, 16, 64
NC = 8
CHUNK = 512
NCH = S // CHUNK       # chunks per batch: 4
NKB = S // 128         # key blocks per batch: 16
QT = S // CHUNK        # q tiles per batch: 4
EPS = 1.1920929e-07
THETA = 10000.0
NKEEP = 12             # PE warm-keeper matmuls bridging the pair-1 A2A

_CACHE = {}
DEBUG_DUMP = False


def _build():
    nc = bacc.Bacc("TRN2", target_bir_lowering=False, debug=False, num_devices=NC)

    xt_d = nc.dram_tensor("xt", [D, S], F32, kind="ExternalInput")
    nw_d = nc.dram_tensor("nw", [D, 1], F32, kind="ExternalInput")
    wq_d = nc.dram_tensor("wqc", [D, 256], F32, kind="ExternalInput")
    wk_d = nc.dram_tensor("wkc", [D, 256], F32, kind="ExternalInput")
    wv_d = nc.dram_tensor("wvc", [D, 256], F32, kind="ExternalInput")
    wo_d = nc.dram_tensor("wo", [D, D], F32, kind="ExternalInput")
    cos_d = nc.dram_tensor("cosb", [128, S], BF16, kind="ExternalInput")
    sin_d = nc.dram_tensor("sinb", [128, S], BF16, kind="ExternalInput")
    msk_d = nc.dram_tensor("dmask", [128, 128], BF16, kind="ExternalInput")
    pm_d = nc.dram_tensor("permm", [128, 128], BF16, kind="ExternalInput")
    id_d = nc.dram_tensor("identm", [128, 128], BF16, kind="ExternalInput")
    out_d = nc.dram_tensor("out", [CHUNK, D], F32, kind="ExternalOutput")

    dmy_in = nc.dram_tensor("dmy_in", [1, 64], BF16)
    dmy_out = nc.dram_tensor("dmy_out", [NC, 64], BF16, addr_space="Shared")
    # per-pair AllToAll: slot j (rows [j*128,(j+1)*128)) = my 128 ctx dims
    # of pair p x rank j's 256-token sub-chunk of my batch; after the
    # exchange, (row i*128+r, col t) holds ctx dim r of pair p of rank i
    # for my token t -- uniform on all cores.  Pair 0's exchange triggers
    # before the final (pair 1) SDPA step and overlaps it.
    a2aA_in = nc.dram_tensor("a2aA_in", [NC * 128, 256], BF16)
    a2aA_out = nc.dram_tensor("a2aA_out", [NC * 128, 256], BF16)
    a2aB_in = nc.dram_tensor("a2aB_in", [NC * 128, 256], BF16)
    a2aB_out = nc.dram_tensor("a2aB_out", [NC * 128, 256], BF16)

    if DEBUG_DUMP:
        dbg_q = nc.dram_tensor("dbg_q", [128, 2 * S], BF16, kind="ExternalOutput")
        dbg_k = nc.dram_tensor("dbg_k", [128, 2 * S], BF16, kind="ExternalOutput")
        dbg_v = nc.dram_tensor("dbg_v", [128, 2 * NKB * 130], BF16,
                               kind="ExternalOutput")
        dbg_c = nc.dram_tensor("dbg_c", [128, 2 * S], BF16, kind="ExternalOutput")
        dbg_i = nc.dram_tensor("dbg_i", [128, 2 * S], BF16, kind="ExternalOutput")
        dbg_ao = nc.dram_tensor("dbg_ao", [NC * 128, 256], BF16,
                                kind="ExternalOutput")

    with tile.TileContext(nc) as tc, ExitStack() as ctx:
        pp = ctx.enter_context(tc.tile_pool(name="persist", bufs=1))

        # ---- persistent tiles ----
        # q/k layout: chunk-major, pair within chunk: col = i*1024 + p*512 + t
        qT = pp.tile([128, 2 * S], BF16, tag="qT")
        kT = pp.tile([128, 2 * S], BF16, tag="kT")
        v_all = pp.tile([128, 2 * NKB * 130], BF16, tag="v_all")
        dmsk = pp.tile([128, 128], BF16, tag="dmsk")   # additive bias: 0 / -240
        identm = pp.tile([128, 128], BF16, tag="identm")
        ones1 = pp.tile([1, 128], BF16, tag="ones1")
        ones128 = pp.tile([128, 1], BF16, tag="ones128")
        nw_sb = pp.tile([128, 8], F32, tag="nw_sb")
        permT = pp.tile([128, 128], BF16, tag="permT")
        wq_sb = pp.tile([128, 8, 256], BF16, tag="wq_sb")
        wk_sb = pp.tile([128, 8, 256], BF16, tag="wk_sb")
        wv_sb = pp.tile([128, 8, 256], BF16, tag="wv_sb")
        wo_sb = pp.tile([128, 8, 1024], BF16, tag="wo_sb")
        ctx_sb = pp.tile([128, 2 * S], BF16, tag="ctx_sb")   # pair p at p*S
        invB_sb = pp.tile([128, 2 * S], BF16, tag="invB_sb")  # chunk i at i*1024
        invT_sb = pp.tile([128, 16], F32, tag="invT_sb")   # chunk i at 4i..4i+4
        eps_t = pp.tile([1, 1], F32, tag="eps_t")
        dmy_sb = pp.tile([1, 64], BF16, tag="dmy_sb")
        vv4 = v_all.rearrange("p (pr blk h c) -> p pr blk h c", pr=2, blk=NKB, c=65)

        nc.vector.memset(ones1, 1.0)
        nc.vector.memset(ones128, 1.0)
        nc.vector.memset(eps_t, float(EPS))
        nc.vector.memset(dmy_sb, 0.0)
        # dummy collective: absorbs the one-time global barrier in the
        # background while phase B computes
        nc.sync.dma_start(out=dmy_in[:, :], in_=dmy_sb)
        nc.gpsimd.collective_compute(
            "AllGather", mybir.AluOpType.bypass,
            replica_groups=[list(range(NC))],
            ins=[dmy_in.ap().opt()], outs=[dmy_out.ap().opt()])
        # ones columns of the extended V blocks (softmax denominator)
        nc.gpsimd.memset(vv4[:, :, :, :, 64:65], 1.0)
        nc.sync.dma_start(out=nw_sb.rearrange("p (kt o) -> p kt o", o=1),
                          in_=nw_d.rearrange("(kt p) o -> p kt o", p=128))

        # ---- pools ----
        sc3 = nc.named_scope("p3_projsdpa"); sc3.__enter__()
        pb_cm = tc.tile_pool(name="pb", bufs=2)
        pb_pool = pb_cm.__enter__()
        pexp_cm = tc.tile_pool(name="pexp", bufs=4)
        pexp = pexp_cm.__enter__()
        cn_cm = tc.tile_pool(name="cn", bufs=2)
        cn = cn_cm.__enter__()
        cs_cm = tc.tile_pool(name="cspool", bufs=2)
        cs_pool = cs_cm.__enter__()
        big_cm = tc.tile_pool(name="bigp", bufs=2, space="PSUM")
        big_pool = big_cm.__enter__()
        ctx_cm = tc.tile_pool(name="ctxp", bufs=1, space="PSUM")
        ctx_pool = ctx_cm.__enter__()
        wo_cm = tc.tile_pool(name="wostage", bufs=2)
        wostage = wo_cm.__enter__()
        xf_cm = tc.tile_pool(name="xf", bufs=8)
        xf_pool = xf_cm.__enter__()
        xc_cm = tc.tile_pool(name="xc", bufs=2)
        xc_pool = xc_cm.__enter__()
        xq_cm = tc.tile_pool(name="xq", bufs=2)
        xq_pool = xq_cm.__enter__()
        ssq_cm = tc.tile_pool(name="ssqp", bufs=1, space="PSUM")
        ssq_pool = ssq_cm.__enter__()

        xcs = {}

        def emit_load(i):
            """DMA + cast + square for chunk i (no PE work)."""
            xc = xc_pool.tile([128, 8, CHUNK], BF16, name="xc", tag="xc")
            xq = xq_pool.tile([128, 8, CHUNK], BF16, name="xq", tag="xq")
            cst = cs_pool.tile([128, 1024], BF16, name="cst", tag="cst")
            snt = cs_pool.tile([128, 1024], BF16, name="snt", tag="snt")
            xcs[i] = (xc, xq, cst, snt)
            for p in (0, 1):
                nc.sync.dma_start(out=cst[:, p * 512 : (p + 1) * 512],
                                  in_=cos_d[:, i * CHUNK : (i + 1) * CHUNK])
                nc.sync.dma_start(out=snt[:, p * 512 : (p + 1) * 512],
                                  in_=sin_d[:, i * CHUNK : (i + 1) * CHUNK])
            for kt in range(8):
                xf = xf_pool.tile([128, CHUNK], F32, name="xf", tag="xf")
                nc.sync.dma_start(
                    out=xf, in_=xt_d[kt * 128 : (kt + 1) * 128,
                                     i * CHUNK : (i + 1) * CHUNK])
                nc.vector.tensor_copy(xc[:, kt, :], xf)
                nc.gpsimd.tensor_mul(xq[:, kt, :], xc[:, kt, :], xc[:, kt, :])

        def stage_w(w_sb, d):
            st = wstage.tile([128, 8 * 256], F32, tag="wst")
            nc.sync.dma_start(out=st.rearrange("p (kt c) -> p kt c", c=256),
                              in_=d.rearrange("(kt p) c -> p kt c", p=128))
            for kt in range(8):
                nc.vector.tensor_scalar(
                    out=w_sb[:, kt, :], in0=st[:, kt * 256 : (kt + 1) * 256],
                    scalar1=nw_sb[:, kt : kt + 1], scalar2=None,
                    op0=mybir.AluOpType.mult)

        def emit_inv(i):
            """ssq-reduce -> inv row (pair-duplicated bcast) + inv column."""
            xq = xcs[i][1]
            ssq = ssq_pool.tile([1, CHUNK], F32, name="ssq", tag="ssq")
            for kt in range(8):
                nc.tensor.matmul(ssq, ones128, xq[:, kt, :],
                                 start=(kt == 0), stop=(kt == 7))
            rstd = cn.tile([1, CHUNK], F32, name="rstd", tag="rstd")
            nc.scalar.activation(rstd, ssq, mybir.ActivationFunctionType.Sqrt,
                                 bias=eps_t[0:1, 0:1], scale=1.0 / D)
            inv = cn.tile([1, CHUNK], F32, name="inv", tag="inv")
            nc.vector.reciprocal_approx_fast(out=inv, in_=rstd)
            invb = cn.tile([1, CHUNK], BF16, name="invb", tag="invb")
            nc.vector.tensor_copy(invb, inv)
            bigt = big_pool.tile([128, 1024], F32, name="ibp", tag="big")
            nc.tensor.matmul(bigt[:, 0:512], ones1, invb, start=True, stop=True)
            nc.tensor.matmul(bigt[:, 512:1024], ones1, invb, start=True, stop=True)
            nc.vector.tensor_copy(invB_sb[:, i * 1024 : (i + 1) * 1024], bigt)
            bigt2 = big_pool.tile([128, 1024], F32, name="itp", tag="big")
            for tb in range(4):
                nc.tensor.matmul(bigt2[:, tb : tb + 1],
                                 invb[0:1, tb * 128 : (tb + 1) * 128],
                                 ones1[0:1, 0:1], start=True, stop=True)
            nc.vector.tensor_copy(invT_sb[:, i * 4 : (i + 1) * 4], bigt2[:, 0:4])

        def proj_chunk(i):
            """all projection matmuls first (PE never waits on DVE), then
            the rope finishes and V scales."""
            xc, _, cst, snt = xcs[i]
            c0 = i * 1024
            accs, ts, swps = {}, {}, {}
            for name, w_sb in (("q", wq_sb), ("k", wk_sb)):
                bigt = big_pool.tile([128, 1024], F32, name="a" + name, tag="big")
                accs[name] = bigt
                for p in (0, 1):
                    acc = bigt[:, p * 512 : (p + 1) * 512]
                    for kt in range(8):
                        nc.tensor.matmul(acc, w_sb[:, kt, p * 128 : (p + 1) * 128],
                                         xc[:, kt, :], start=(kt == 0),
                                         stop=(kt == 7))
                t = pb_pool.tile([128, 1024], BF16, tag="rt")
                nc.vector.tensor_mul(t, bigt,
                                     invB_sb[:, i * 1024 : (i + 1) * 1024])
                ts[name] = t
            for name in ("q", "k"):
                swpt = big_pool.tile([128, 1024], F32, name="s" + name, tag="big")
                swps[name] = swpt
                t = ts[name]
                nc.tensor.matmul(swpt[:, 0:512], permT, t[:, 0:512],
                                 start=True, stop=True)
                nc.tensor.matmul(swpt[:, 512:1024], permT, t[:, 512:1024],
                                 start=True, stop=True)
            # V token-major: [128 tokens, 256 dims] per 128-token block, all
            # four blocks in one PSUM tile
            vtile = big_pool.tile([128, 1024], F32, name="vb", tag="big")
            for tb in range(4):
                vp = vtile[:, tb * 256 : (tb + 1) * 256]
                for kt in range(8):
                    nc.tensor.matmul(
                        vp, xc[:, kt, tb * 128 : (tb + 1) * 128],
                        wv_sb[:, kt, :], start=(kt == 0), stop=(kt == 7))
            # rope finishes (DVE) -- overlap the V matmuls
            for name, dst in (("q", qT), ("k", kT)):
                t, swpt = ts[name], swps[name]
                sw = pb_pool.tile([128, 1024], BF16, tag="rsw")
                nc.vector.tensor_mul(sw, swpt, snt)
                nc.vector.tensor_mul(t, t, cst)
                nc.vector.tensor_add(dst[:, c0 : c0 + 1024], t, sw)
            # V scale per token block (per-partition scalar)
            for tb in range(4):
                blk = i * 4 + tb
                vp = vtile[:, tb * 256 : (tb + 1) * 256]
                src = vp.rearrange("p (pr h c) -> p pr h c", pr=2, c=64)
                nc.vector.tensor_scalar(
                    out=vv4[:, :, blk, :, 0:64], in0=src,
                    scalar1=invT_sb[:, blk : blk + 1], scalar2=None,
                    op0=mybir.AluOpType.mult)

        def emit_norm(p, j, ctxp):
            base = p * S
            for hi, r0 in ((0, 0), (1, 64)):
                den_s = cn.tile([1, CHUNK], F32, tag="den_s")
                nc.vector.tensor_copy(den_s, ctxp[hi][64:65, :])
                rec = cn.tile([1, CHUNK], F32, tag="rec")
                nc.vector.reciprocal_approx_fast(out=rec, in_=den_s)
                recb = cn.tile([1, CHUNK], BF16, tag="recb")
                nc.vector.tensor_copy(recb, rec)
                bigt = big_pool.tile([128, 1024], F32, name="bc", tag="big")
                nc.tensor.matmul(bigt[0:64, 0:CHUNK], ones1[0:1, 0:64], recb,
                                 start=True, stop=True)
                bcs = cn.tile([64, CHUNK], BF16, tag="bcs")
                nc.vector.tensor_copy(bcs, bigt[0:64, 0:CHUNK])
                nc.vector.tensor_mul(
                    ctx_sb[r0 : r0 + 64, base + CHUNK * j : base + CHUNK * (j + 1)],
                    ctxp[hi][0:64, :], bcs)
            a2a_d = a2aA_in if p == 0 else a2aB_in
            for half in (0, 1):
                slot = (j * 2 + half) * 128
                col0 = base + j * CHUNK + half * 256
                nc.sync.dma_start(
                    out=a2a_d[slot : slot + 128, :],
                    in_=ctx_sb[:, col0 : col0 + 256])

        def emit_block(p, j, kb, ctxp, sc_pools=None):
            nkb = 4 * (j + 1)
            m = kb - 4 * j
            c0 = 128 * m if m >= 0 else 0
            w = CHUNK - c0
            qcol0 = j * 1024 + p * 512 + c0
            koff = (kb // 4) * 1024 + p * 512 + (kb % 4) * 128
            pool = big_pool if sc_pools is None else sc_pools[kb % len(sc_pools)]
            sc = pool.tile([128, 1024], F32, name="sc", tag="big")
            for hi, r0 in ((0, 0), (1, 64)):
                nc.tensor.matmul(
                    sc[:, hi * 512 + c0 : hi * 512 + 512],
                    kT[r0 : r0 + 64, koff : koff + 128],
                    qT[r0 : r0 + 64, qcol0 : qcol0 + w],
                    start=True, stop=(m < 0), skip_group_check=True)
                if m >= 0:
                    # diag block: accumulate the causal bias (-240 above the
                    # diagonal) into the 128-col window via identity matmul
                    nc.tensor.matmul(
                        sc[:, hi * 512 + c0 : hi * 512 + c0 + 128],
                        identm, dmsk, start=False, stop=True,
                        skip_group_check=True)
            pt = pexp.tile([128, 1024], BF16, name="pt", tag="pt")
            nc.scalar.activation(pt[:, c0:1024], sc[:, c0:1024],
                                 mybir.ActivationFunctionType.Exp, scale=0.125)
            vcol = (p * NKB + kb) * 130
            for hi in range(2):
                nc.tensor.matmul(
                    ctxp[hi][:, c0:CHUNK],
                    v_all[:, vcol + hi * 65 : vcol + hi * 65 + 65],
                    pt[:, hi * 512 + c0 : hi * 512 + 512],
                    start=(kb == 0), stop=(kb == nkb - 1),
                    skip_group_check=True)

        def emit_step(p, j, sc_pools=None):
            ctxp = {0: ctx_pool.tile([65, CHUNK], F32, name="ctxA", tag="ctxA"),
                    1: ctx_pool.tile([65, CHUNK], F32, name="ctxB", tag="ctxB")}
            for kb in range(4 * (j + 1)):
                emit_block(p, j, kb, ctxp, sc_pools)
            emit_norm(p, j, ctxp)

        def stage_wo(kt):
            st2 = wostage.tile([128, 1024], F32, tag="wost")
            nc.sync.dma_start(out=st2[0:64, :], in_=wo_d[kt * 128 : kt * 128 + 64, :])
            nc.sync.dma_start(out=st2[64:128, :],
                              in_=wo_d[kt * 128 + 64 : (kt + 1) * 128, :])
            nc.gpsimd.tensor_copy(wo_sb[:, kt, :], st2)

        # ---- startup: x chunk 0 first, then weights/tables by need-time ----
        emit_load(0)
        wst_cm = tc.tile_pool(name="wstage", bufs=2)
        wstage = wst_cm.__enter__()
        stage_w(wq_sb, wq_d)
        stage_w(wk_sb, wk_d)
        stage_w(wv_sb, wv_d)
        wst_cm.__exit__(None, None, None)
        nc.sync.dma_start(out=dmsk, in_=msk_d[:, :])
        nc.sync.dma_start(out=permT, in_=pm_d[:, :])
        nc.sync.dma_start(out=identm, in_=id_d[:, :])

        # steps run as their chunk prerequisites complete; chunk i+1's
        # load + inv chain is emitted inside chunk i's step stream so the
        # Sqrt/recip/broadcast latency hides behind the step matmuls
        steps_by_chunk = {0: [(0, 0)], 1: [(1, 0), (0, 1)],
                          2: [(1, 1), (0, 2)], 3: [(1, 2), (0, 3)]}
        emit_inv(0)
        for i in range(NCH):
            proj_chunk(i)
            stage_wo(2 * i)
            stage_wo(2 * i + 1)
            if i + 1 < NCH:
                # chunk i+1's load + inv chain ahead of this chunk's steps:
                # its Sqrt enters the ACT queue before the step exps, and
                # its casts/squares/ssq run under the proj/step matmuls
                emit_load(i + 1)
                emit_inv(i + 1)
            for (p, j) in steps_by_chunk[i]:
                emit_step(p, j)

        # pair-0 context is complete: launch its AllToAll now so the wire
        # time hides behind the final (pair 1) step
        sc4 = nc.named_scope("p4_a2aA"); sc4.__enter__()
        nc.gpsimd.collective_compute(
            "AllToAll", mybir.AluOpType.bypass,
            replica_groups=[list(range(NC))],
            ins=[a2aA_in.ap().opt()], outs=[a2aA_out.ap().opt()])
        sc4.__exit__(None, None, None)

        # release the x/ssq pools, run the final step with a 3-deep
        # score-buffer rotation (big,big,sc3)
        ssq_cm.__exit__(None, None, None)
        xq_cm.__exit__(None, None, None)
        xc_cm.__exit__(None, None, None)
        xf_cm.__exit__(None, None, None)
        sc3p_cm = tc.tile_pool(name="sc3p", bufs=1, space="PSUM")
        sc3p = sc3p_cm.__enter__()
        emit_step(1, 3, sc_pools=[big_pool, big_pool, sc3p])

        sc3p_cm.__exit__(None, None, None)
        wo_cm.__exit__(None, None, None)
        ctx_cm.__exit__(None, None, None)
        big_cm.__exit__(None, None, None)
        cs_cm.__exit__(None, None, None)
        cn_cm.__exit__(None, None, None)
        pexp_cm.__exit__(None, None, None)
        pb_cm.__exit__(None, None, None)
        sc3.__exit__(None, None, None)

        # ---- A2A (pair 1) + out-projection ----
        # ctx dim-tile u (0..7) = pair u%2 of group-rank u//2 -> rows
        # (h*4 + u//2)*128 of a2a{A,B}_out for batch h
        sc6 = nc.named_scope("p6_outproj"); sc6.__enter__()
        with tc.tile_pool(name="ctxgp", bufs=1) as ctxgp, \
             tc.tile_pool(name="outp", bufs=2) as outp, \
             tc.tile_pool(name="ps6", bufs=2, space="PSUM") as ps6, \
             tc.tile_pool(name="keep", bufs=1, space="PSUM") as keepp:
            sc5 = nc.named_scope("p5_a2aB"); sc5.__enter__()
            nc.gpsimd.collective_compute(
                "AllToAll", mybir.AluOpType.bypass,
                replica_groups=[list(range(NC))],
                ins=[a2aB_in.ap().opt()], outs=[a2aB_out.ap().opt()])
            sc5.__exit__(None, None, None)

            ctxg = ctxgp.tile([128, 8, 2, 256], BF16, tag="ctxg")
            for pr, a2a_o in ((0, a2aA_out), (1, a2aB_out)):
                for h in range(2):
                    for gi in range(4):
                        r0 = (h * 4 + gi) * 128
                        nc.sync.dma_start(out=ctxg[:, 2 * gi + pr, h, :],
                                          in_=a2a_o[r0 : r0 + 128, :])

            def op_mms(pso, h, tb2, us, start, stop):
                for n, u in enumerate(us):
                    stat = ctxg[:, u, h, tb2 * 128 : (tb2 + 1) * 128]
                    st = start and n == 0
                    sp = stop and n == len(us) - 1
                    nc.tensor.matmul(pso[:, 0:512], stat, wo_sb[:, u, 0:512],
                                     start=st, stop=sp, skip_group_check=True)
                    nc.tensor.matmul(pso[:, 512:1024], stat,
                                     wo_sb[:, u, 512:1024],
                                     start=st, stop=sp, skip_group_check=True)

            def flush(pso, h, tb2):
                ost = outp.tile([128, 1024], F32, tag="ost")
                nc.scalar.copy(ost[:, 0:512], pso[:, 0:512])
                nc.scalar.copy(ost[:, 512:1024], pso[:, 512:1024])
                r0 = h * 256 + tb2 * 128
                nc.sync.dma_start(out=out_d[r0 : r0 + 64, :], in_=ost[0:64, :])
                nc.sync.dma_start(out=out_d[r0 + 64 : r0 + 128, :],
                                  in_=ost[64:128, :])

            # batch-0 halves: pair-0 contributions run during the pair-1
            # A2A (warm-keeping the PE), then keepers bridge the rest
            pso0 = {tb2: ps6.tile([128, 1024], F32, name=f"ps0{tb2}", tag="pso")
                    for tb2 in range(2)}
            for tb2 in range(2):
                op_mms(pso0[tb2], 0, tb2, [0, 2, 4, 6], True, False)
            keep = keepp.tile([128, 512], F32, tag="keep")
            for _ in range(NKEEP):
                nc.tensor.matmul(keep, wo_sb[:, 0, 0:128], wo_sb[:, 1, 0:512],
                                 start=True, stop=True)
            for tb2 in range(2):
                op_mms(pso0[tb2], 0, tb2, [1, 3, 5, 7], False, True)
                flush(pso0[tb2], 0, tb2)
            for tb2 in range(2):
                pso = ps6.tile([128, 1024], F32, name=f"ps1{tb2}", tag="pso")
                op_mms(pso, 1, tb2, [0, 2, 4, 6, 1, 3, 5, 7], True, True)
                flush(pso, 1, tb2)
        sc6.__exit__(None, None, None)

        if DEBUG_DUMP:
            nc.sync.dma_start(out=dbg_q[:, :], in_=qT)
            nc.sync.dma_start(out=dbg_k[:, :], in_=kT)
            nc.sync.dma_start(out=dbg_v[:, :], in_=v_all)
            nc.sync.dma_start(out=dbg_c[:, :], in_=ctx_sb)
            nc.sync.dma_start(out=dbg_i[:, :], in_=invB_sb)
            nc.sync.dma_start(out=dbg_ao[:, :], in_=a2aA_out[:, :])

    nc.compile()
    return nc


def _head_cols(h, deinterleave):
    base = h * DH
    if deinterleave:
        return np.concatenate([base + np.arange(0, DH, 2), base + np.arange(1, DH, 2)])
    return base + np.arange(DH)


def _make_tables():
    inv_freq = 1.0 / (THETA ** (np.arange(0, DH, 2) / DH))   # [32]
    ang = np.arange(S)[:, None] * inv_freq[None, :]          # [2048, 32]
    ch = np.cos(ang).T.astype(np.float32)                    # [32, 2048]
    sh = np.sin(ang).T.astype(np.float32)
    cosb = np.concatenate([ch, ch, ch, ch], axis=0)          # [128, 2048]
    sinb = np.concatenate([-sh, sh, -sh, sh], axis=0)
    kk, qq = np.meshgrid(np.arange(128), np.arange(128), indexing="ij")
    dmask = np.where(kk <= qq, 0.0, -240.0).astype(np.float32)
    bf = ml_dtypes.bfloat16
    return cosb.astype(bf), sinb.astype(bf), dmask.astype(bf)


def _in_maps(inputs):
    x = np.ascontiguousarray(inputs["x"], dtype=np.float32)
    norm_w = np.asarray(inputs["norm_w"], dtype=np.float32)
    wq = np.asarray(inputs["wq"], dtype=np.float32)
    wk = np.asarray(inputs["wk"], dtype=np.float32)
    wv = np.asarray(inputs["wv"], dtype=np.float32)
    wo = np.ascontiguousarray(inputs["wo"], dtype=np.float32)

    xT = [np.ascontiguousarray(x[b].T) for b in range(B)]    # [1024, 2048] each
    cosb, sinb, dmask = _make_tables()
    nw = np.ascontiguousarray(norm_w.reshape(D, 1))
    # rotate-half partition swap as a (symmetric) permutation matrix
    swap = np.concatenate([np.arange(32, 64), np.arange(0, 32),
                           np.arange(96, 128), np.arange(64, 96)])
    perm = np.zeros((128, 128), np.float32)
    perm[np.arange(128), swap] = 1.0
    perm = perm.astype(ml_dtypes.bfloat16)
    ident = np.eye(128, dtype=np.float32).astype(ml_dtypes.bfloat16)

    maps = []
    for c in range(NC):
        b, g = c // 4, c % 4
        heads = [4 * g + 0, 4 * g + 1, 4 * g + 2, 4 * g + 3]
        qcols = np.concatenate([_head_cols(h, True) for h in heads])
        vcols = np.concatenate([_head_cols(h, False) for h in heads])
        maps.append({
            "xt": xT[b],
            "nw": nw,
            "wqc": np.ascontiguousarray(wq[:, qcols]),
            "wkc": np.ascontiguousarray(wk[:, qcols]),
            "wvc": np.ascontiguousarray(wv[:, vcols]),
            "wo": wo,
            "cosb": cosb,
            "sinb": sinb,
            "dmask": dmask,
            "permm": perm,
            "identm": ident,
        })
    return maps


def _run(inputs, trace=False):
    if "ncs" not in _CACHE:
        _CACHE["ncs"] = _build()
    nc = _CACHE["ncs"]
    res = run_bass_kernel_spmd(nc, _in_maps(inputs), core_ids=list(range(NC)),
                               trace=trace)
    out = np.empty((B, S, D), dtype=np.float32)
    for c in range(NC):
        chunk = res.results[c]["out"]        # [512, D]: rows 0-255 batch 0,
        out[0, c * 256 : (c + 1) * 256] = chunk[0:256]    # 256-511 batch 1
        out[1, c * 256 : (c + 1) * 256] = chunk[256:512]
    return out, res


def kernel(**inputs) -> np.ndarray:
    out, _ = _run(inputs, trace=False)
    return out
